# revision 22
# baseline (speedup 1.0000x reference)
# Braak-aware attention kernel for Trainium2 (Bass/Tile), 8 NeuronCores.
#
# Problem (per sample b of B=8, all fp32 in HBM):
#   bias[s]   = braak_embed[braak_stages[b], s]          (per-row constant)
#   q'[s,d]   = query[b,s,d] + bias[s]
#   S[s,t]    = sum_d q'[s,d] * key[b,t,d]
#   P         = softmax_t(S)
#   out[s,d]  = sum_t P[s,t] * value[b,t,d]
#
# Sharding: data-parallel, one sample per core (8 samples, 8 cores), no comms.
# The braak_embed gather by integer stage is host-side (pure indexing); the
# bias ADD happens on-device.
#
# Device strategy (per core):
#   - Q tile: +bias and cast fp32->fp16 fused in one DVE tensor_scalar pass,
#     then PE-transposed (fp16 transpose-mode) to Q'^T [d,s]; prefetched one
#     s-tile ahead of the scores that consume it.
#   - K tile: fp32 PE-transpose straight from the DMA-staged tile; the fp16
#     cast rides the PSUM->SBUF copy (split between ACT and DVE).
#   - scores: fp16 matmul S = (Q'^T).T @ K^T accumulated fp32 in PSUM.
#   - softmax: DVE reduce_max(negate) -> ACT Exp(bias=-max) with fused
#     accum_out row-sum; P stays fp32. Normalization deferred to the output.
#   - P^T via fp32 PE transposes; out = (P^T).T @ V with both operands
#     bitcast to float32r (full-rate PE at N=512, no V cast pass), then one
#     DVE tensor_scalar_mul by 1/rowsum on the PSUM->SBUF copy, DMA out.
#   - identity matrices are host inputs (no gpsimd in the startup path);
#     dummy transposes on the identity warm the PE HAM clock during the
#     initial DMA wait.

import os
import sys

for _p in ("/opt/trn_rl_repo",):
    if _p not in sys.path:
        sys.path.insert(0, _p)

import numpy as np

import concourse.bass as bass
import concourse.tile as tile
from concourse import bacc, mybir
from concourse.bass_utils import run_bass_kernel_spmd

B, S, D = 8, 1024, 1024
P = 128
NT = S // P  # 8 row tiles per matrix
F32 = mybir.dt.float32
F32R = mybir.dt.float32r
F16 = mybir.dt.float16
EXP = mybir.ActivationFunctionType.Exp

N_WARMUP = 40  # dummy PE transposes to lift the HAM clock gate at start

_CACHE = {}


def _build(ctx, tc):
    nc = tc.nc
    q_d = nc.dram_tensor("q", [S, D], F32, kind="ExternalInput").ap()
    k_d = nc.dram_tensor("k", [S, D], F32, kind="ExternalInput").ap()
    v_d = nc.dram_tensor("v", [S, D], F32, kind="ExternalInput").ap()
    bias_d = nc.dram_tensor("bias", [S], F32, kind="ExternalInput").ap()
    id16_d = nc.dram_tensor("ident16", [P, P], F16, kind="ExternalInput").ap()
    id32_d = nc.dram_tensor("ident32", [P, P], F32, kind="ExternalInput").ap()
    out_d = nc.dram_tensor("out", [S, D], F32, kind="ExternalOutput").ap()

    const = ctx.enter_context(tc.tile_pool(name="const", bufs=1))
    wts = ctx.enter_context(tc.tile_pool(name="wts", bufs=1))
    stage = ctx.enter_context(tc.tile_pool(name="stage", bufs=3))
    nat16 = ctx.enter_context(tc.tile_pool(name="nat16", bufs=2))
    ppool = ctx.enter_context(tc.tile_pool(name="ppool", bufs=2))
    ptpool = ctx.enter_context(tc.tile_pool(name="ptpool", bufs=2))
    outpool = ctx.enter_context(tc.tile_pool(name="outpool", bufs=2))
    smalls = ctx.enter_context(tc.tile_pool(name="smalls", bufs=2))
    psum_s = ctx.enter_context(tc.tile_pool(name="psum_s", bufs=2, space="PSUM"))
    psum_tp = ctx.enter_context(tc.tile_pool(name="psum_tp", bufs=2, space="PSUM"))
    psum_o = ctx.enter_context(tc.tile_pool(name="psum_o", bufs=1, space="PSUM"))

    ident = const.tile([P, P], F16, tag="ident")
    nc.sync.dma_start(out=ident, in_=id16_d)
    ident32 = const.tile([P, P], F32, tag="ident32")
    nc.sync.dma_start(out=ident32, in_=id32_d)
    # bias_sb[p, i] = bias[i*128 + p]: per-partition scalar column per s-tile
    bias_sb = const.tile([P, NT], F32, tag="bias")
    nc.sync.dma_start(out=bias_sb, in_=bias_d.rearrange("(i p) -> p i", p=P))

    # Persistent operands: [128, tile_idx, 1024]
    khT = wts.tile([P, NT, S], F16, tag="khT")  # [d_in_tile, d_tile k, t]
    qhT = wts.tile([P, NT, S], F16, tag="qhT")  # [d_in_tile, d_tile k, s]
    vf = wts.tile([P, NT, D], F16, tag="vf")  # [t_in_tile, t_tile j, d]

    # ---- PE warmup: keep the systolic array busy >3.4us so HAM un-gates ----
    for g in range(0, N_WARMUP, 4):
        wtp = psum_tp.tile([P, 4 * P], F16, tag="tp", name="warm")
        for m in range(4):
            nc.tensor.matmul(
                wtp[:, m * P : (m + 1) * P],
                ident,
                ident,
                is_transpose=True,
                start=(m == 0),
                stop=(m == 3),
            )

    # ---- K phase: load fp32, PE-transpose, cast on PSUM->SBUF copy ----
    # copies alternate ACT/DVE so neither engine serializes the phase
    def k_phase():
        for j in range(NT):
            kst = stage.tile([P, D], F32, tag="kload", name="kst", bufs=6)
            nc.sync.dma_start(out=kst, in_=k_d[j * P : (j + 1) * P, :])
            for g in range(2):
                tp = psum_tp.tile([P, 4 * P], F32, tag="tp", name="ktp")
                for m in range(4):
                    blk = g * 4 + m
                    nc.tensor.matmul(
                        tp[:, m * P : (m + 1) * P],
                        kst[:, blk * P : (blk + 1) * P],
                        ident32,
                        is_transpose=True,
                        start=(m == 0),
                        stop=(m == 3),
                    )
                dst = khT[:, g * 4 : (g + 1) * 4, j * P : (j + 1) * P]
                src = tp.rearrange("p (k s) -> p k s", k=4)
                if (j * 2 + g) % 2 == 0:
                    nc.scalar.copy(out=dst, in_=src)
                else:
                    nc.vector.tensor_copy(out=dst, in_=src)

    # ---- V phase: load fp32, cast fp16; casts split ACT/DVE ----
    def v_phase():
        for j in range(NT):
            vst = stage.tile([P, D], F32, tag="vload", name="vst", bufs=4)
            nc.sync.dma_start(out=vst, in_=v_d[j * P : (j + 1) * P, :])
            if j % 2 == 0:
                nc.scalar.copy(out=vf[:, j, :], in_=vst)
            else:
                nc.vector.tensor_copy(out=vf[:, j, :], in_=vst)

    # ---- per-s-tile stages ----
    def stage_front(i):
        """Load Q tile i, add bias + cast fp16 (DVE), PE-transpose, ACT copy."""
        qst = stage.tile([P, D], F32, tag="qload", name="qst")
        nc.sync.dma_start(out=qst, in_=q_d[i * P : (i + 1) * P, :])
        qnat = nat16.tile([P, D], F16, tag="qnat", name="qnat")
        nc.vector.tensor_scalar_add(out=qnat, in0=qst, scalar1=bias_sb[:, i : i + 1])
        qtp = psum_tp.tile([P, NT * P], F16, tag="tp", name="qtp")
        for m in range(NT):
            nc.tensor.matmul(
                qtp[:, m * P : (m + 1) * P],
                qnat[:, m * P : (m + 1) * P],
                ident,
                is_transpose=True,
                start=(m == 0),
                stop=(m == NT - 1),
            )
        nc.scalar.copy(
            out=qhT[:, :, i * P : (i + 1) * P],
            in_=qtp.rearrange("p (k s) -> p k s", k=NT),
        )

    def stage_scores(i):
        sp = psum_s.tile([P, S], F32, tag="sp", name="sp")
        for k in range(NT):
            lhsT = qhT[:, k, i * P : (i + 1) * P]
            for h in range(2):
                nc.tensor.matmul(
                    sp[:, h * 512 : (h + 1) * 512],
                    lhsT,
                    khT[:, k, h * 512 : (h + 1) * 512],
                    start=(k == 0),
                    stop=(k == NT - 1),
                )
        return sp

    def stage_softmax(i, sp):
        negmax = smalls.tile([P, 1], F32, tag="negmax", name="negmax")
        nc.vector.reduce_max(
            out=negmax, in_=sp, axis=mybir.AxisListType.X, negate=True
        )
        pexp = ppool.tile([P, S], F16, tag="pexp", name="pexp")
        sumexp = smalls.tile([P, 1], F32, tag="sumexp", name="sumexp")
        nc.scalar.activation(
            out=pexp, in_=sp, func=EXP, bias=negmax, scale=1.0, accum_out=sumexp
        )
        recip = smalls.tile([P, 1], F32, tag="recip", name="recip")
        nc.vector.reciprocal(out=recip, in_=sumexp)
        return pexp, recip

    def stage_pt(i, pexp):
        """Transpose P (fp16, one PSUM bank), copy to SBUF."""
        ptp = psum_tp.tile([P, NT * P], F16, tag="tp", name="ptp")
        for m in range(NT):
            nc.tensor.matmul(
                ptp[:, m * P : (m + 1) * P],
                pexp[:, m * P : (m + 1) * P],
                ident,
                is_transpose=True,
                start=(m == 0),
                stop=(m == NT - 1),
            )
        pt = ptpool.tile([P, NT * P], F16, tag="pt", name="pt")
        nc.scalar.copy(out=pt, in_=ptp)
        return pt

    def stage_av(i, pt, recip):
        op = psum_o.tile([P, D], F32, tag="op", name="op")
        for j in range(NT):
            lhsT = pt[:, j * P : (j + 1) * P]
            for h in range(2):
                nc.tensor.matmul(
                    op[:, h * 512 : (h + 1) * 512],
                    lhsT,
                    vf[:, j, h * 512 : (h + 1) * 512],
                    start=(j == 0),
                    stop=(j == NT - 1),
                )
        ot = outpool.tile([P, D], F32, tag="ot", name="ot")
        nc.vector.tensor_scalar_mul(out=ot, in0=op, scalar1=recip)
        nc.sync.dma_start(out=out_d[i * P : (i + 1) * P, :], in_=ot)

    # ---- schedule ----
    k_phase()
    stage_front(0)
    state = {}
    prev = None
    for i in range(NT):
        if i + 1 < NT:
            stage_front(i + 1)  # Q path prefetched one iteration ahead
        if prev is not None:
            state["pt"] = stage_pt(prev, state["pexp"])
        sp = stage_scores(i)
        state_sm = stage_softmax(i, sp)
        if i == 0:
            v_phase()  # V loads/casts issue behind softmax(0)
        if prev is not None:
            stage_av(prev, state["pt"], state["recip"])
        state["pexp"], state["recip"] = state_sm
        prev = i
    state["pt"] = stage_pt(prev, state["pexp"])
    stage_av(prev, state["pt"], state["recip"])


def _get_program():
    key = "v3"
    if key not in _CACHE:
        nc = bacc.Bacc("TRN2", num_devices=B)
        from contextlib import ExitStack

        with tile.TileContext(nc) as tc:
            with ExitStack() as ctx:
                _build(ctx, tc)
        nc.compile()
        _CACHE[key] = nc
    return _CACHE[key]


def kernel(query, key, value, braak_embed, braak_stages):
    query = np.ascontiguousarray(np.asarray(query, dtype=np.float32))
    key_in = np.ascontiguousarray(np.asarray(key, dtype=np.float32))
    value = np.ascontiguousarray(np.asarray(value, dtype=np.float32))
    braak_embed = np.asarray(braak_embed, dtype=np.float32)
    stages = np.asarray(braak_stages).astype(np.int64)

    bias = braak_embed[stages]  # [B, S] host-side gather (pure indexing)
    id16 = np.eye(P, dtype=np.float16)
    id32 = np.eye(P, dtype=np.float32)

    nc = _get_program()
    in_maps = [
        {
            "q": query[b],
            "k": key_in[b],
            "v": value[b],
            "bias": np.ascontiguousarray(bias[b]),
            "ident16": id16,
            "ident32": id32,
        }
        for b in range(B)
    ]
    trace = os.environ.get("BRAAK_TRACE", "0") == "1"
    res = run_bass_kernel_spmd(nc, in_maps, list(range(B)), trace=trace)
    if trace:
        kernel.last_exec_time_ns = res.exec_time_ns
        kernel.last_profile = res
    out = np.stack([res.results[b]["out"] for b in range(B)]).astype(np.float32)
    return out


kernel.last_exec_time_ns = None
kernel.last_profile = None
